# revision 17
# baseline (speedup 1.0000x reference)
"""Trainium2 Bass kernel for nn_Encoder segment-reduce.

Reference computation (per sample b):
    cls = onehot(argmax_k outputs[b])            # [K, HW]
    sizes = cls.sum(HW) + 0.01                   # [K]
    feat_set = feats[b] @ cls.T / sizes          # [F, K]
    out[b] = w_proj @ feat_set + bias            # [E, K]

Kernel strategy (pure data parallel: 1 sample per NeuronCore, 8 cores):
    Since the division by sizes and the projection are both linear, reorder:
        out[b].T[k, e] = (onehot.T @ (feats.T @ wT))[k, e] / sizes[k] + bias[e]
    The inner matmul projT[hw, e] = feats_chunk.T @ wT uses feats tiles as the
    matmul's STATIONARY operand in their natural [F, HW] layout, so no
    transpose of the 32MB feats tensor is ever needed.  The segment-reduce
    then contracts projT (hw on partitions) against the onehot matrix
    (hw on partitions), accumulating [K, E+2] in PSUM across all hw chunks —
    the two extra `ones` columns appended to projT make the same matmul
    accumulate the class sizes for free.

    argmax one-hot: PE-transpose outputs chunks [K,128] -> [128,K], then
    rowmax (DVE reduce) + is_equal compare.

dtype: "f32r" (full fp32 DMA, float32r full-rate matmuls, rel err ~2e-4) or
"bf16" (host-cast feats/wT to bf16: half the HBM traffic, rel err ~5e-3).
"""

import numpy as np

import concourse.bacc as bacc
import concourse.bass as bass
import concourse.mybir as mybir
import concourse.tile as tile
from concourse.bass import ds, ts
from concourse.bass_utils import run_bass_kernel_spmd
from concourse.masks import make_identity

# Problem shapes (hardcoded per contract)
B = 8
K = 21
H = 64
W = 64
HW = H * W            # 4096
F = 2048
E = 256
P = 128
FC = F // P           # 16 f-chunks
N_CORES = 8

F32 = mybir.dt.float32
F32R = mybir.dt.float32r
BF16 = mybir.dt.bfloat16

DTYPE = "bf16"        # "bf16" or "f32r"
HW_BLK = 512          # hw columns per feats block (host layout must match)
N_BLK = HW // HW_BLK


def build_module(dtype=DTYPE, hw_blk=HW_BLK, feats_bufs=4):
    n_blk = HW // hw_blk
    sub = hw_blk // P
    n_t = HW // P

    mm_dt = BF16 if dtype == "bf16" else F32R
    nc = bacc.Bacc("TRN2", target_bir_lowering=False, debug=False)

    outputs_d = nc.dram_tensor("outputs_in", [K, HW], F32, kind="ExternalInput")
    # feats is host-reshuffled to [p, g, fc, hw_blk] so each partition's
    # per-block DMA source run is fc*hw_blk contiguous bytes.
    feats_d = nc.dram_tensor(
        "feats_in", [P, n_blk, FC, hw_blk], mm_dt, kind="ExternalInput"
    )
    wT_d = nc.dram_tensor("wT_in", [F, E], mm_dt, kind="ExternalInput")
    bias_d = nc.dram_tensor("bias_in", [E], F32, kind="ExternalInput")
    out_d = nc.dram_tensor("out", [K, E], F32, kind="ExternalOutput")

    with tile.TileContext(nc) as tc:
        with (
            tc.tile_pool(name="consts", bufs=1) as consts,
            tc.tile_pool(name="feats", bufs=feats_bufs) as feats_pool,
            tc.tile_pool(name="small", bufs=4) as small,
            tc.tile_pool(name="projT", bufs=3) as projT_pool,
            tc.tile_pool(name="outp", bufs=1) as outp,
            tc.tile_pool(name="ps_tr", bufs=2, space="PSUM") as ps_tr,
            tc.tile_pool(name="ps_proj", bufs=4, space="PSUM") as ps_proj,
            tc.tile_pool(name="ps_out", bufs=1, space="PSUM") as ps_out_pool,
        ):
            # outputs first on the sync HWDGE queue (phase 1 needs it ASAP);
            # feats blocks follow on the same queue.  wT/bias ride the gpsimd
            # SWDGE queue so they land in parallel with the feats stream.
            outputs_sb = consts.tile([K, HW], F32)
            nc.sync.dma_start(out=outputs_sb, in_=outputs_d.ap())

            feats_r = feats_d.ap()
            fgs = []
            for g in range(n_blk):
                fg = feats_pool.tile([P, FC, hw_blk], mm_dt)
                nc.sync.dma_start(out=fg, in_=feats_r[:, g])
                fgs.append(fg)

            ident = consts.tile([P, P], F32)
            make_identity(nc, ident)
            ones_f = consts.tile([P, 2], F32)
            nc.vector.memset(ones_f, 1.0)

            wT_sb = consts.tile([P, FC, E], mm_dt)
            nc.gpsimd.dma_start(
                out=wT_sb, in_=wT_d.ap().rearrange("(fc p) e -> p fc e", p=P)
            )
            bias_ap = bias_d.ap()
            bias_bc = consts.tile([K, E], F32)
            nc.gpsimd.dma_start(
                out=bias_bc,
                in_=bass.AP(
                    tensor=bias_ap.tensor, offset=bias_ap.offset, ap=[[0, K], [1, E]]
                ),
            )

            # psum_out columns [0:E) accumulate onehot.T @ projT; columns
            # [E:E+2) accumulate onehot.T @ 1 = the class sizes.
            psum_out = ps_out_pool.tile([K, E + 2], F32)
            oh_all = consts.tile([P, n_t, K], mm_dt)

            # Phase 1: onehot construction
            for t in range(n_t):
                tr = ps_tr.tile([P, K], F32)
                nc.tensor.transpose(tr, outputs_sb[:, ts(t, P)], ident[:K, :K])
                rowmax = small.tile([P, 1], F32)
                nc.vector.tensor_reduce(
                    rowmax, tr, mybir.AxisListType.X, mybir.AluOpType.max
                )
                nc.vector.tensor_scalar(
                    out=oh_all[:, t, :],
                    in0=tr,
                    scalar1=rowmax,
                    scalar2=None,
                    op0=mybir.AluOpType.is_equal,
                )

            # Phase 2: projection (feats stationary) + segment accumulate
            for g in range(n_blk):
                fg = fgs[g]
                for s in range(sub):
                    t = g * sub + s
                    pt = ps_proj.tile([P, E], F32)
                    for fc in range(FC):
                        nc.tensor.matmul(
                            pt,
                            lhsT=fg[:, fc, ts(s, P)],
                            rhs=wT_sb[:, fc, :],
                            start=(fc == 0),
                            stop=(fc == FC - 1),
                        )
                    pts = projT_pool.tile([P, E + 2], mm_dt)
                    nc.vector.tensor_copy(pts[:, 0:E], pt)
                    nc.vector.tensor_copy(pts[:, E : E + 2], ones_f)
                    nc.tensor.matmul(
                        psum_out,
                        lhsT=oh_all[:, t, :],
                        rhs=pts,
                        start=(t == 0),
                        stop=(t == n_t - 1),
                    )

            # Phase 3: scale by 1/sizes, add bias, store
            sizes_sb = small.tile([K, 1], F32, tag="sizes")
            nc.vector.tensor_scalar_add(sizes_sb, psum_out[:, E : E + 1], 0.01)
            recip = small.tile([K, 1], F32, tag="recip")
            nc.vector.reciprocal(recip, sizes_sb)
            out_sb = outp.tile([K, E], F32)
            nc.vector.scalar_tensor_tensor(
                out=out_sb,
                in0=psum_out[:, 0:E],
                scalar=recip,
                in1=bias_bc,
                op0=mybir.AluOpType.mult,
                op1=mybir.AluOpType.add,
            )
            nc.sync.dma_start(out=out_d.ap(), in_=out_sb)

    nc.compile()
    return nc


_CACHE = {}


def make_in_maps(outputs, feats, w_proj, b_proj, dtype=DTYPE):
    import ml_dtypes

    mm_np = ml_dtypes.bfloat16 if dtype == "bf16" else np.float32
    outputs = np.ascontiguousarray(np.asarray(outputs, dtype=np.float32))
    feats = np.asarray(feats, dtype=np.float32).astype(mm_np)
    # [B, F, H, W] -> per sample [p, g, fc, hw_blk] block-major layout
    feats_sh = np.ascontiguousarray(
        feats.reshape(B, FC, P, N_BLK, HW_BLK).transpose(0, 2, 3, 1, 4)
    )
    wT = np.ascontiguousarray(np.asarray(w_proj, dtype=np.float32).T.astype(mm_np))
    bias = np.ascontiguousarray(np.asarray(b_proj, dtype=np.float32))
    return [
        {
            "outputs_in": outputs[b].reshape(K, HW),
            "feats_in": feats_sh[b],
            "wT_in": wT,
            "bias_in": bias,
        }
        for b in range(B)
    ]


def kernel(outputs, feats, w_proj, b_proj, _trace=False, _trace_kwargs=None,
           _dtype=DTYPE, _build_kwargs=None):
    key = (_dtype, tuple(sorted((_build_kwargs or {}).items())))
    if key not in _CACHE:
        _CACHE[key] = build_module(dtype=_dtype, **(_build_kwargs or {}))
    nc = _CACHE[key]
    in_maps = make_in_maps(outputs, feats, w_proj, b_proj, dtype=_dtype)
    res = run_bass_kernel_spmd(
        nc,
        in_maps,
        core_ids=list(range(N_CORES)),
        trace=_trace,
        **(_trace_kwargs or {}),
    )
    out = np.stack([np.asarray(r["out"]).T for r in res.results])
    if _trace:
        _CACHE["last_results"] = res
    return out


# revision 20
# speedup vs baseline: 1.0433x; 1.0433x over previous
"""Trainium2 Bass kernel for nn_Encoder segment-reduce.

Reference computation (per sample b):
    cls = onehot(argmax_k outputs[b])            # [K, HW]
    sizes = cls.sum(HW) + 0.01                   # [K]
    feat_set = feats[b] @ cls.T / sizes          # [F, K]
    out[b] = w_proj @ feat_set + bias            # [E, K]

Kernel strategy (pure data parallel: 1 sample per NeuronCore, 8 cores):
    Since the division by sizes and the projection are both linear, reorder:
        out[b].T[k, e] = (onehot.T @ (feats.T @ wT))[k, e] / sizes[k] + bias[e]
    The inner matmul projT[hw, e] = feats_chunk.T @ wT uses feats tiles as the
    matmul's STATIONARY operand in their natural [F, HW] layout, so no
    transpose of the 32MB feats tensor is ever needed.  The segment-reduce
    then contracts projT (hw on partitions) against the onehot matrix
    (hw on partitions), accumulating [K, E+2] in PSUM across all hw chunks —
    the two extra `ones` columns appended to projT make the same matmul
    accumulate the class sizes for free.

    argmax one-hot: PE-transpose outputs chunks [K,128] -> [128,K], then
    rowmax (DVE reduce) + is_equal compare.

dtype: "f32r" (full fp32 DMA, float32r full-rate matmuls, rel err ~2e-4) or
"bf16" (host-cast feats/wT to bf16: half the HBM traffic, rel err ~5e-3).
"""

import numpy as np

import concourse.bacc as bacc
import concourse.bass as bass
import concourse.mybir as mybir
import concourse.tile as tile
from concourse.bass import ds, ts
from concourse.bass_utils import run_bass_kernel_spmd
from concourse.masks import make_identity

# Problem shapes (hardcoded per contract)
B = 8
K = 21
H = 64
W = 64
HW = H * W            # 4096
F = 2048
E = 256
P = 128
FC = F // P           # 16 f-chunks
N_CORES = 8

F32 = mybir.dt.float32
F32R = mybir.dt.float32r
BF16 = mybir.dt.bfloat16

DTYPE = "bf16"        # "bf16" or "f32r"
HW_BLK = 512          # hw columns per feats block (host layout must match)
N_BLK = HW // HW_BLK


def build_module(dtype=DTYPE, hw_blk=HW_BLK, feats_bufs=6):
    n_blk = HW // hw_blk
    sub = hw_blk // P
    n_t = HW // P

    mm_dt = BF16 if dtype == "bf16" else F32R
    nc = bacc.Bacc("TRN2", target_bir_lowering=False, debug=False)

    outputs_d = nc.dram_tensor("outputs_in", [K, HW], F32, kind="ExternalInput")
    # feats is host-reshuffled to [p, g, fc, hw_blk] so each partition's
    # per-block DMA source run is fc*hw_blk contiguous bytes.
    feats_d = nc.dram_tensor(
        "feats_in", [P, n_blk, FC, hw_blk], mm_dt, kind="ExternalInput"
    )
    wT_d = nc.dram_tensor("wT_in", [F, E], mm_dt, kind="ExternalInput")
    bias_d = nc.dram_tensor("bias_in", [E], F32, kind="ExternalInput")
    out_d = nc.dram_tensor("out", [K, E], F32, kind="ExternalOutput")

    with tile.TileContext(nc) as tc:
        with (
            tc.tile_pool(name="consts", bufs=1) as consts,
            tc.tile_pool(name="feats", bufs=feats_bufs) as feats_pool,
            tc.tile_pool(name="small", bufs=4) as small,
            tc.tile_pool(name="projT", bufs=3) as projT_pool,
            tc.tile_pool(name="outp", bufs=1) as outp,
            tc.tile_pool(name="ps_tr", bufs=2, space="PSUM") as ps_tr,
            tc.tile_pool(name="ps_proj", bufs=5, space="PSUM") as ps_proj,
            tc.tile_pool(name="ps_out", bufs=1, space="PSUM") as ps_out_pool,
        ):
            # All bulk DMAs ride the sync HWDGE queue (FIFO): wT first (needed
            # by the first projection matmul), then outputs (phase 1), then
            # the feats block stream.  bias rides the gpsimd SWDGE queue.
            wT_sb = consts.tile([P, FC, E], mm_dt)
            nc.sync.dma_start(
                out=wT_sb, in_=wT_d.ap().rearrange("(fc p) e -> p fc e", p=P)
            )
            outputs_sb = consts.tile([K, HW], F32)
            nc.sync.dma_start(out=outputs_sb, in_=outputs_d.ap())

            feats_r = feats_d.ap()
            fgs = []
            for g in range(n_blk):
                fg = feats_pool.tile([P, FC, hw_blk], mm_dt)
                nc.sync.dma_start(out=fg, in_=feats_r[:, g])
                fgs.append(fg)

            ident = consts.tile([P, P], F32)
            make_identity(nc, ident)
            ones_f = consts.tile([P, 2], F32)
            nc.vector.memset(ones_f, 1.0)

            bias_ap = bias_d.ap()
            bias_bc = consts.tile([K, E], F32)
            nc.gpsimd.dma_start(
                out=bias_bc,
                in_=bass.AP(
                    tensor=bias_ap.tensor, offset=bias_ap.offset, ap=[[0, K], [1, E]]
                ),
            )

            # psum_out columns [0:E) accumulate onehot.T @ projT; columns
            # [E:E+2) accumulate onehot.T @ 1 = the class sizes.
            psum_out = ps_out_pool.tile([K, E + 2], F32)
            oh_all = consts.tile([P, n_t, K], mm_dt)

            # Phase 1: onehot construction
            for t in range(n_t):
                tr = ps_tr.tile([P, K], F32)
                nc.tensor.transpose(tr, outputs_sb[:, ts(t, P)], ident[:K, :K])
                rowmax = small.tile([P, 1], F32)
                nc.vector.tensor_reduce(
                    rowmax, tr, mybir.AxisListType.X, mybir.AluOpType.max
                )
                nc.vector.tensor_scalar(
                    out=oh_all[:, t, :],
                    in0=tr,
                    scalar1=rowmax,
                    scalar2=None,
                    op0=mybir.AluOpType.is_equal,
                )

            # Phase 2: projection (feats stationary) + segment accumulate
            for g in range(n_blk):
                fg = fgs[g]
                for s in range(sub):
                    t = g * sub + s
                    pt = ps_proj.tile([P, E], F32)
                    for fc in range(FC):
                        nc.tensor.matmul(
                            pt,
                            lhsT=fg[:, fc, ts(s, P)],
                            rhs=wT_sb[:, fc, :],
                            start=(fc == 0),
                            stop=(fc == FC - 1),
                        )
                    pts = projT_pool.tile([P, E + 2], mm_dt)
                    nc.vector.tensor_copy(pts[:, 0:E], pt)
                    nc.vector.tensor_copy(pts[:, E : E + 2], ones_f)
                    nc.tensor.matmul(
                        psum_out,
                        lhsT=oh_all[:, t, :],
                        rhs=pts,
                        start=(t == 0),
                        stop=(t == n_t - 1),
                    )

            # Phase 3: scale by 1/sizes, add bias, store
            sizes_sb = small.tile([K, 1], F32, tag="sizes")
            nc.vector.tensor_scalar_add(sizes_sb, psum_out[:, E : E + 1], 0.01)
            recip = small.tile([K, 1], F32, tag="recip")
            nc.vector.reciprocal(recip, sizes_sb)
            out_sb = outp.tile([K, E], F32)
            nc.vector.scalar_tensor_tensor(
                out=out_sb,
                in0=psum_out[:, 0:E],
                scalar=recip,
                in1=bias_bc,
                op0=mybir.AluOpType.mult,
                op1=mybir.AluOpType.add,
            )
            nc.sync.dma_start(out=out_d.ap(), in_=out_sb)

    nc.compile()
    return nc


_CACHE = {}


def make_in_maps(outputs, feats, w_proj, b_proj, dtype=DTYPE):
    import ml_dtypes

    mm_np = ml_dtypes.bfloat16 if dtype == "bf16" else np.float32
    outputs = np.ascontiguousarray(np.asarray(outputs, dtype=np.float32))
    feats = np.asarray(feats, dtype=np.float32).astype(mm_np)
    # [B, F, H, W] -> per sample [p, g, fc, hw_blk] block-major layout
    feats_sh = np.ascontiguousarray(
        feats.reshape(B, FC, P, N_BLK, HW_BLK).transpose(0, 2, 3, 1, 4)
    )
    wT = np.ascontiguousarray(np.asarray(w_proj, dtype=np.float32).T.astype(mm_np))
    bias = np.ascontiguousarray(np.asarray(b_proj, dtype=np.float32))
    return [
        {
            "outputs_in": outputs[b].reshape(K, HW),
            "feats_in": feats_sh[b],
            "wT_in": wT,
            "bias_in": bias,
        }
        for b in range(B)
    ]


def kernel(outputs, feats, w_proj, b_proj, _trace=False, _trace_kwargs=None,
           _dtype=DTYPE, _build_kwargs=None):
    key = (_dtype, tuple(sorted((_build_kwargs or {}).items())))
    if key not in _CACHE:
        _CACHE[key] = build_module(dtype=_dtype, **(_build_kwargs or {}))
    nc = _CACHE[key]
    in_maps = make_in_maps(outputs, feats, w_proj, b_proj, dtype=_dtype)
    res = run_bass_kernel_spmd(
        nc,
        in_maps,
        core_ids=list(range(N_CORES)),
        trace=_trace,
        **(_trace_kwargs or {}),
    )
    out = np.stack([np.asarray(r["out"]).T for r in res.results])
    if _trace:
        _CACHE["last_results"] = res
    return out


# revision 21
# speedup vs baseline: 1.0931x; 1.0477x over previous
"""Trainium2 Bass kernel for nn_Encoder segment-reduce.

Reference computation (per sample b):
    cls = onehot(argmax_k outputs[b])            # [K, HW]
    sizes = cls.sum(HW) + 0.01                   # [K]
    feat_set = feats[b] @ cls.T / sizes          # [F, K]
    out[b] = w_proj @ feat_set + bias            # [E, K]

Kernel strategy (pure data parallel: 1 sample per NeuronCore, 8 cores):
    Since the division by sizes and the projection are both linear, reorder:
        out[b].T[k, e] = (onehot.T @ (feats.T @ wT))[k, e] / sizes[k] + bias[e]
    The inner matmul projT[hw, e] = feats_chunk.T @ wT uses feats tiles as the
    matmul's STATIONARY operand in their natural [F, HW] layout, so no
    transpose of the 32MB feats tensor is ever needed.  The segment-reduce
    then contracts projT (hw on partitions) against the onehot matrix
    (hw on partitions), accumulating [K, E+2] in PSUM across all hw chunks —
    the two extra `ones` columns appended to projT make the same matmul
    accumulate the class sizes for free.

    argmax one-hot: PE-transpose outputs chunks [K,128] -> [128,K], then
    rowmax (DVE reduce) + is_equal compare.

dtype: "f32r" (full fp32 DMA, float32r full-rate matmuls, rel err ~2e-4) or
"bf16" (host-cast feats/wT to bf16: half the HBM traffic, rel err ~5e-3).
"""

import numpy as np

import concourse.bacc as bacc
import concourse.bass as bass
import concourse.mybir as mybir
import concourse.tile as tile
from concourse.bass import ds, ts
from concourse.bass_utils import run_bass_kernel_spmd
from concourse.masks import make_identity

# Problem shapes (hardcoded per contract)
B = 8
K = 21
H = 64
W = 64
HW = H * W            # 4096
F = 2048
E = 256
P = 128
FC = F // P           # 16 f-chunks
N_CORES = 8

F32 = mybir.dt.float32
F32R = mybir.dt.float32r
BF16 = mybir.dt.bfloat16

DTYPE = "bf16"        # "bf16" or "f32r"
HW_BLK = 512          # hw columns per feats block (host layout must match)
N_BLK = HW // HW_BLK


def build_module(dtype=DTYPE, hw_blk=HW_BLK, feats_bufs=6):
    n_blk = HW // hw_blk
    sub = hw_blk // P
    n_t = HW // P

    mm_dt = BF16 if dtype == "bf16" else F32R
    nc = bacc.Bacc("TRN2", target_bir_lowering=False, debug=False)

    outputs_d = nc.dram_tensor("outputs_in", [K, HW], F32, kind="ExternalInput")
    # feats is host-reshuffled to [p, g, fc, hw_blk] so each partition's
    # per-block DMA source run is fc*hw_blk contiguous bytes.
    feats_d = nc.dram_tensor(
        "feats_in", [P, n_blk, FC, hw_blk], mm_dt, kind="ExternalInput"
    )
    wT_d = nc.dram_tensor("wT_in", [F, E], mm_dt, kind="ExternalInput")
    bias_d = nc.dram_tensor("bias_in", [E], F32, kind="ExternalInput")
    out_d = nc.dram_tensor("out", [K, E], F32, kind="ExternalOutput")

    with tile.TileContext(nc) as tc:
        with (
            tc.tile_pool(name="consts", bufs=1) as consts,
            tc.tile_pool(name="feats", bufs=feats_bufs) as feats_pool,
            tc.tile_pool(name="small", bufs=4) as small,
            tc.tile_pool(name="projT", bufs=3) as projT_pool,
            tc.tile_pool(name="outp", bufs=1) as outp,
            tc.tile_pool(name="ps_tr", bufs=2, space="PSUM") as ps_tr,
            tc.tile_pool(name="ps_proj", bufs=5, space="PSUM") as ps_proj,
            tc.tile_pool(name="ps_out", bufs=1, space="PSUM") as ps_out_pool,
        ):
            # All bulk DMAs ride the sync HWDGE queue (FIFO): wT first (needed
            # by the first projection matmul), then outputs (phase 1), then
            # the feats block stream.  bias rides the gpsimd SWDGE queue.
            wT_sb = consts.tile([P, FC, E], mm_dt)
            nc.sync.dma_start(
                out=wT_sb, in_=wT_d.ap().rearrange("(fc p) e -> p fc e", p=P)
            )
            outputs_sb = consts.tile([K, HW], F32)
            nc.sync.dma_start(out=outputs_sb, in_=outputs_d.ap())

            feats_r = feats_d.ap()
            fgs = []
            for g in range(n_blk):
                fg = feats_pool.tile([P, FC, hw_blk], mm_dt)
                nc.sync.dma_start(out=fg, in_=feats_r[:, g])
                fgs.append(fg)

            ident = consts.tile([P, P], F32)
            make_identity(nc, ident)
            ones_f = consts.tile([P, 2], F32)
            nc.vector.memset(ones_f, 1.0)

            bias_ap = bias_d.ap()
            bias_bc = consts.tile([K, E], F32)
            nc.gpsimd.dma_start(
                out=bias_bc,
                in_=bass.AP(
                    tensor=bias_ap.tensor, offset=bias_ap.offset, ap=[[0, K], [1, E]]
                ),
            )

            # psum_out columns [0:E) accumulate onehot.T @ projT; columns
            # [E:E+2) accumulate onehot.T @ 1 = the class sizes.
            psum_out = ps_out_pool.tile([K, E + 2], F32)
            oh_all = consts.tile([P, n_t, K], mm_dt)

            LOOKAHEAD = 2

            def make_onehot(t):
                # onehot chunk t: PE-transpose outputs[:, t*128:(t+1)*128],
                # then rowmax + is_equal on DVE.
                tr = ps_tr.tile([P, K], F32)
                nc.tensor.transpose(tr, outputs_sb[:, ts(t, P)], ident[:K, :K])
                rowmax = small.tile([P, 1], F32)
                nc.vector.tensor_reduce(
                    rowmax, tr, mybir.AxisListType.X, mybir.AluOpType.max
                )
                nc.vector.tensor_scalar(
                    out=oh_all[:, t, :],
                    in0=tr,
                    scalar1=rowmax,
                    scalar2=None,
                    op0=mybir.AluOpType.is_equal,
                )

            # The onehot construction is interleaved into the projection
            # stream LOOKAHEAD iterations ahead of the segment matmul that
            # consumes it, so the DVE round-trip hides under proj matmuls.
            for t in range(LOOKAHEAD):
                make_onehot(t)

            for g in range(n_blk):
                fg = fgs[g]
                for s in range(sub):
                    t = g * sub + s
                    if t + LOOKAHEAD < n_t:
                        make_onehot(t + LOOKAHEAD)
                    pt = ps_proj.tile([P, E], F32)
                    for fc in range(FC):
                        nc.tensor.matmul(
                            pt,
                            lhsT=fg[:, fc, ts(s, P)],
                            rhs=wT_sb[:, fc, :],
                            start=(fc == 0),
                            stop=(fc == FC - 1),
                        )
                    pts = projT_pool.tile([P, E + 2], mm_dt)
                    nc.vector.tensor_copy(pts[:, E : E + 2], ones_f)
                    nc.vector.tensor_copy(pts[:, 0:E], pt)
                    nc.tensor.matmul(
                        psum_out,
                        lhsT=oh_all[:, t, :],
                        rhs=pts,
                        start=(t == 0),
                        stop=(t == n_t - 1),
                    )

            # Phase 3: scale by 1/sizes, add bias, store
            sizes_sb = small.tile([K, 1], F32, tag="sizes")
            nc.vector.tensor_scalar_add(sizes_sb, psum_out[:, E : E + 1], 0.01)
            recip = small.tile([K, 1], F32, tag="recip")
            nc.vector.reciprocal(recip, sizes_sb)
            out_sb = outp.tile([K, E], F32)
            nc.vector.scalar_tensor_tensor(
                out=out_sb,
                in0=psum_out[:, 0:E],
                scalar=recip,
                in1=bias_bc,
                op0=mybir.AluOpType.mult,
                op1=mybir.AluOpType.add,
            )
            nc.sync.dma_start(out=out_d.ap(), in_=out_sb)

    nc.compile()
    return nc


_CACHE = {}


def make_in_maps(outputs, feats, w_proj, b_proj, dtype=DTYPE):
    import ml_dtypes

    mm_np = ml_dtypes.bfloat16 if dtype == "bf16" else np.float32
    outputs = np.ascontiguousarray(np.asarray(outputs, dtype=np.float32))
    feats = np.asarray(feats, dtype=np.float32).astype(mm_np)
    # [B, F, H, W] -> per sample [p, g, fc, hw_blk] block-major layout
    feats_sh = np.ascontiguousarray(
        feats.reshape(B, FC, P, N_BLK, HW_BLK).transpose(0, 2, 3, 1, 4)
    )
    wT = np.ascontiguousarray(np.asarray(w_proj, dtype=np.float32).T.astype(mm_np))
    bias = np.ascontiguousarray(np.asarray(b_proj, dtype=np.float32))
    return [
        {
            "outputs_in": outputs[b].reshape(K, HW),
            "feats_in": feats_sh[b],
            "wT_in": wT,
            "bias_in": bias,
        }
        for b in range(B)
    ]


def kernel(outputs, feats, w_proj, b_proj, _trace=False, _trace_kwargs=None,
           _dtype=DTYPE, _build_kwargs=None):
    key = (_dtype, tuple(sorted((_build_kwargs or {}).items())))
    if key not in _CACHE:
        _CACHE[key] = build_module(dtype=_dtype, **(_build_kwargs or {}))
    nc = _CACHE[key]
    in_maps = make_in_maps(outputs, feats, w_proj, b_proj, dtype=_dtype)
    res = run_bass_kernel_spmd(
        nc,
        in_maps,
        core_ids=list(range(N_CORES)),
        trace=_trace,
        **(_trace_kwargs or {}),
    )
    out = np.stack([np.asarray(r["out"]).T for r in res.results])
    if _trace:
        _CACHE["last_results"] = res
    return out


# revision 24
# speedup vs baseline: 1.1176x; 1.0224x over previous
"""Trainium2 Bass kernel for nn_Encoder segment-reduce.

Reference computation (per sample b):
    cls = onehot(argmax_k outputs[b])            # [K, HW]
    sizes = cls.sum(HW) + 0.01                   # [K]
    feat_set = feats[b] @ cls.T / sizes          # [F, K]
    out[b] = w_proj @ feat_set + bias            # [E, K]

Kernel strategy (pure data parallel: 1 sample per NeuronCore, 8 cores):
    Since the division by sizes and the projection are both linear, reorder:
        out[b].T[k, e] = (onehot.T @ (feats.T @ wT))[k, e] / sizes[k] + bias[e]
    The inner matmul projT[hw, e] = feats_chunk.T @ wT uses feats tiles as the
    matmul's STATIONARY operand in their natural [F, HW] layout, so no
    transpose of the 32MB feats tensor is ever needed.  The segment-reduce
    then contracts projT (hw on partitions) against the onehot matrix
    (hw on partitions), accumulating [K, E+2] in PSUM across all hw chunks —
    the two extra `ones` columns appended to projT make the same matmul
    accumulate the class sizes for free.

    argmax one-hot: PE-transpose outputs chunks [K,128] -> [128,K], then
    rowmax (DVE reduce) + is_equal compare.

dtype: "f32r" (full fp32 DMA, float32r full-rate matmuls, rel err ~2e-4) or
"bf16" (host-cast feats/wT to bf16: half the HBM traffic, rel err ~5e-3).
"""

import numpy as np

import concourse.bacc as bacc
import concourse.bass as bass
import concourse.mybir as mybir
import concourse.tile as tile
from concourse.bass import ds, ts
from concourse.bass_utils import run_bass_kernel_spmd
from concourse.masks import make_identity

# Problem shapes (hardcoded per contract)
B = 8
K = 21
H = 64
W = 64
HW = H * W            # 4096
F = 2048
E = 256
P = 128
FC = F // P           # 16 f-chunks
N_CORES = 8

F32 = mybir.dt.float32
F32R = mybir.dt.float32r
BF16 = mybir.dt.bfloat16

DTYPE = "bf16"        # "bf16" or "f32r"
HW_BLK = 512          # hw columns per feats block (host layout must match)
N_BLK = HW // HW_BLK


def build_module(dtype=DTYPE, hw_blk=HW_BLK, feats_bufs=6):
    n_blk = HW // hw_blk
    sub = hw_blk // P
    n_t = HW // P

    mm_dt = BF16 if dtype == "bf16" else F32R
    nc = bacc.Bacc("TRN2", target_bir_lowering=False, debug=False)

    outputs_d = nc.dram_tensor("outputs_in", [K, HW], F32, kind="ExternalInput")
    # feats is host-reshuffled to [p, g, fc, hw_blk] so each partition's
    # per-block DMA source run is fc*hw_blk contiguous bytes.
    feats_d = nc.dram_tensor(
        "feats_in", [P, n_blk, FC, hw_blk], mm_dt, kind="ExternalInput"
    )
    wT_d = nc.dram_tensor("wT_in", [F, E], mm_dt, kind="ExternalInput")
    bias_d = nc.dram_tensor("bias_in", [E], F32, kind="ExternalInput")
    out_d = nc.dram_tensor("out", [K, E], F32, kind="ExternalOutput")

    with tile.TileContext(nc) as tc:
        with (
            tc.tile_pool(name="consts", bufs=1) as consts,
            tc.tile_pool(name="feats", bufs=feats_bufs) as feats_pool,
            tc.tile_pool(name="small", bufs=4) as small,
            tc.tile_pool(name="projT", bufs=3) as projT_pool,
            tc.tile_pool(name="outp", bufs=1) as outp,
            tc.tile_pool(name="ps_tr", bufs=2, space="PSUM") as ps_tr,
            tc.tile_pool(name="ps_proj", bufs=4, space="PSUM") as ps_proj,
            tc.tile_pool(name="ps_out", bufs=1, space="PSUM") as ps_out_pool,
            tc.tile_pool(name="ps_warm", bufs=1, space="PSUM") as ps_warm,
        ):
            # All bulk DMAs ride the sync HWDGE queue (FIFO): outputs first
            # (phase 1 needs it earliest), then wT (first projection), then
            # the feats block stream.  bias rides the gpsimd SWDGE queue.
            outputs_sb = consts.tile([K, HW], F32)
            nc.sync.dma_start(out=outputs_sb, in_=outputs_d.ap())
            wT_sb = consts.tile([P, FC, E], mm_dt)
            nc.sync.dma_start(
                out=wT_sb, in_=wT_d.ap().rearrange("(fc p) e -> p fc e", p=P)
            )

            # PE warm-up: the HAM clock gate holds the PE at 1.2 GHz until it
            # has been busy ~3.4us.  A burst of dummy matmuls (no DMA deps)
            # fills the initial DMA wait so the real stream starts warm.
            warm_w = consts.tile([P, 64], BF16)
            nc.vector.memset(warm_w, 0.0)
            warm_ps = ps_warm.tile([P, 64], F32)
            for _ in range(120):
                nc.tensor.matmul(warm_ps[0:64, :], lhsT=warm_w, rhs=warm_w)

            feats_r = feats_d.ap()
            fgs = []
            for g in range(n_blk):
                fg = feats_pool.tile([P, FC, hw_blk], mm_dt)
                nc.sync.dma_start(out=fg, in_=feats_r[:, g])
                fgs.append(fg)

            ident = consts.tile([P, P], F32)
            make_identity(nc, ident)
            ones_f = consts.tile([P, 2], F32)
            nc.vector.memset(ones_f, 1.0)

            bias_ap = bias_d.ap()
            bias_bc = consts.tile([K, E], F32)
            nc.gpsimd.dma_start(
                out=bias_bc,
                in_=bass.AP(
                    tensor=bias_ap.tensor, offset=bias_ap.offset, ap=[[0, K], [1, E]]
                ),
            )

            # psum_out columns [0:E) accumulate onehot.T @ projT; columns
            # [E:E+2) accumulate onehot.T @ 1 = the class sizes.
            psum_out = ps_out_pool.tile([K, E + 2], F32)
            oh_all = consts.tile([P, n_t, K], mm_dt)

            LOOKAHEAD = 2

            def make_onehot(t):
                # onehot chunk t: PE-transpose outputs[:, t*128:(t+1)*128],
                # then rowmax + is_equal on DVE.
                tr = ps_tr.tile([P, K], F32)
                nc.tensor.transpose(tr, outputs_sb[:, ts(t, P)], ident[:K, :K])
                rowmax = small.tile([P, 1], F32)
                nc.vector.tensor_reduce(
                    rowmax, tr, mybir.AxisListType.X, mybir.AluOpType.max
                )
                nc.vector.tensor_scalar(
                    out=oh_all[:, t, :],
                    in0=tr,
                    scalar1=rowmax,
                    scalar2=None,
                    op0=mybir.AluOpType.is_equal,
                )

            # The onehot construction is interleaved into the projection
            # stream LOOKAHEAD iterations ahead of the segment matmul that
            # consumes it, so the DVE round-trip hides under proj matmuls.
            for t in range(LOOKAHEAD):
                make_onehot(t)

            for g in range(n_blk):
                fg = fgs[g]
                for s in range(sub):
                    t = g * sub + s
                    if t + LOOKAHEAD < n_t:
                        make_onehot(t + LOOKAHEAD)
                    pt = ps_proj.tile([P, E], F32)
                    for fc in range(FC):
                        nc.tensor.matmul(
                            pt,
                            lhsT=fg[:, fc, ts(s, P)],
                            rhs=wT_sb[:, fc, :],
                            start=(fc == 0),
                            stop=(fc == FC - 1),
                        )
                    pts = projT_pool.tile([P, E + 2], mm_dt)
                    nc.vector.tensor_copy(pts[:, E : E + 2], ones_f)
                    nc.vector.tensor_copy(pts[:, 0:E], pt)
                    nc.tensor.matmul(
                        psum_out,
                        lhsT=oh_all[:, t, :],
                        rhs=pts,
                        start=(t == 0),
                        stop=(t == n_t - 1),
                    )

            # Phase 3: scale by 1/sizes, add bias, store
            sizes_sb = small.tile([K, 1], F32, tag="sizes")
            nc.vector.tensor_scalar_add(sizes_sb, psum_out[:, E : E + 1], 0.01)
            recip = small.tile([K, 1], F32, tag="recip")
            nc.vector.reciprocal(recip, sizes_sb)
            out_sb = outp.tile([K, E], F32)
            nc.vector.scalar_tensor_tensor(
                out=out_sb,
                in0=psum_out[:, 0:E],
                scalar=recip,
                in1=bias_bc,
                op0=mybir.AluOpType.mult,
                op1=mybir.AluOpType.add,
            )
            nc.sync.dma_start(out=out_d.ap(), in_=out_sb)

    nc.compile()
    return nc


_CACHE = {}


def make_in_maps(outputs, feats, w_proj, b_proj, dtype=DTYPE):
    import ml_dtypes

    mm_np = ml_dtypes.bfloat16 if dtype == "bf16" else np.float32
    outputs = np.ascontiguousarray(np.asarray(outputs, dtype=np.float32))
    feats = np.asarray(feats, dtype=np.float32).astype(mm_np)
    # [B, F, H, W] -> per sample [p, g, fc, hw_blk] block-major layout
    feats_sh = np.ascontiguousarray(
        feats.reshape(B, FC, P, N_BLK, HW_BLK).transpose(0, 2, 3, 1, 4)
    )
    wT = np.ascontiguousarray(np.asarray(w_proj, dtype=np.float32).T.astype(mm_np))
    bias = np.ascontiguousarray(np.asarray(b_proj, dtype=np.float32))
    return [
        {
            "outputs_in": outputs[b].reshape(K, HW),
            "feats_in": feats_sh[b],
            "wT_in": wT,
            "bias_in": bias,
        }
        for b in range(B)
    ]


def kernel(outputs, feats, w_proj, b_proj, _trace=False, _trace_kwargs=None,
           _dtype=DTYPE, _build_kwargs=None):
    key = (_dtype, tuple(sorted((_build_kwargs or {}).items())))
    if key not in _CACHE:
        _CACHE[key] = build_module(dtype=_dtype, **(_build_kwargs or {}))
    nc = _CACHE[key]
    in_maps = make_in_maps(outputs, feats, w_proj, b_proj, dtype=_dtype)
    res = run_bass_kernel_spmd(
        nc,
        in_maps,
        core_ids=list(range(N_CORES)),
        trace=_trace,
        **(_trace_kwargs or {}),
    )
    out = np.stack([np.asarray(r["out"]).T for r in res.results])
    if _trace:
        _CACHE["last_results"] = res
    return out


# revision 32
# speedup vs baseline: 1.1754x; 1.0518x over previous
"""Trainium2 Bass kernel for nn_Encoder segment-reduce.

Reference computation (per sample b):
    cls = onehot(argmax_k outputs[b])            # [K, HW]
    sizes = cls.sum(HW) + 0.01                   # [K]
    feat_set = feats[b] @ cls.T / sizes          # [F, K]
    out[b] = w_proj @ feat_set + bias            # [E, K]

Kernel strategy (pure data parallel: 1 sample per NeuronCore, 8 cores):
    Since the division by sizes and the projection are both linear, reorder:
        out[b].T[k, e] = (onehot.T @ (feats.T @ wT))[k, e] / sizes[k] + bias[e]
    The inner matmul projT[hw, e] = feats_chunk.T @ wT uses feats tiles as the
    matmul's STATIONARY operand in their natural [F, HW] layout, so no
    transpose of the 32MB feats tensor is ever needed.  The segment-reduce
    then contracts projT (hw on partitions) against the onehot matrix
    (hw on partitions), accumulating [K, E+2] in PSUM across all hw chunks —
    the two extra `ones` columns appended to projT make the same matmul
    accumulate the class sizes for free.

    argmax one-hot: PE-transpose outputs chunks [K,128] -> [128,K], then
    rowmax (DVE reduce) + is_equal compare.

dtype: "f32r" (full fp32 DMA, float32r full-rate matmuls, rel err ~2e-4) or
"bf16" (host-cast feats/wT to bf16: half the HBM traffic, rel err ~5e-3).
"""

import numpy as np

import concourse.bacc as bacc
import concourse.bass as bass
import concourse.mybir as mybir
import concourse.tile as tile
from concourse.bass import ds, ts
from concourse.bass_utils import run_bass_kernel_spmd
from concourse.masks import make_identity

# Problem shapes (hardcoded per contract)
B = 8
K = 21
H = 64
W = 64
HW = H * W            # 4096
F = 2048
E = 256
P = 128
FC = F // P           # 16 f-chunks
N_CORES = 8

F32 = mybir.dt.float32
F32R = mybir.dt.float32r
BF16 = mybir.dt.bfloat16

DTYPE = "bf16"        # "bf16" or "f32r"
HW_BLK = 512          # hw columns per feats block (host layout must match)
N_BLK = HW // HW_BLK


def build_module(dtype=DTYPE, hw_blk=HW_BLK, feats_bufs=6):
    n_blk = HW // hw_blk
    sub = hw_blk // P
    n_t = HW // P

    mm_dt = BF16 if dtype == "bf16" else F32R
    nc = bacc.Bacc("TRN2", target_bir_lowering=False, debug=False)

    # outputs is host-transposed to [p, t, k] (pixel-major) so the argmax
    # runs along the free dim with 128 partitions and no PE transposes.
    outputs_d = nc.dram_tensor("outputs_in", [P, n_t, K], F32, kind="ExternalInput")
    # feats is host-reshuffled to [p, g, fc, hw_blk] so each partition's
    # per-block DMA source run is fc*hw_blk contiguous bytes.
    feats_d = nc.dram_tensor(
        "feats_in", [P, n_blk, FC, hw_blk], mm_dt, kind="ExternalInput"
    )
    wT_d = nc.dram_tensor("wT_in", [F, E], mm_dt, kind="ExternalInput")
    bias_d = nc.dram_tensor("bias_in", [E], F32, kind="ExternalInput")
    out_d = nc.dram_tensor("out", [K, E], F32, kind="ExternalOutput")

    with tile.TileContext(nc) as tc:
        with (
            tc.tile_pool(name="consts", bufs=1) as consts,
            tc.tile_pool(name="feats", bufs=feats_bufs) as feats_pool,
            tc.tile_pool(name="small", bufs=4) as small,
            tc.tile_pool(name="projT", bufs=3) as projT_pool,
            tc.tile_pool(name="outp", bufs=1) as outp,
            tc.tile_pool(name="ps_proj", bufs=5, space="PSUM") as ps_proj,
            tc.tile_pool(name="ps_out", bufs=1, space="PSUM") as ps_out_pool,
            tc.tile_pool(name="ps_warm", bufs=1, space="PSUM") as ps_warm,
        ):
            # All bulk DMAs ride the sync HWDGE queue (FIFO): outputs first
            # (phase 1 needs it earliest), then wT (first projection), then
            # the feats block stream.  bias rides the gpsimd SWDGE queue.
            outputs_sb = consts.tile([P, n_t, K], F32)
            nc.sync.dma_start(out=outputs_sb, in_=outputs_d.ap())
            wT_sb = consts.tile([P, FC, E], mm_dt)
            nc.sync.dma_start(
                out=wT_sb, in_=wT_d.ap().rearrange("(fc p) e -> p fc e", p=P)
            )

            # PE warm-up: the HAM clock gate holds the PE at 1.2 GHz until it
            # has been busy ~3.4us.  A burst of dummy matmuls (no DMA deps)
            # fills the initial DMA wait so the real stream starts warm.
            warm_w = consts.tile([P, 64], BF16)
            nc.vector.memset(warm_w, 0.0)
            warm_ps = ps_warm.tile([P, 64], F32)
            for _ in range(150):
                nc.tensor.matmul(warm_ps[0:64, :], lhsT=warm_w, rhs=warm_w)

            feats_r = feats_d.ap()
            fgs = []
            for g in range(n_blk):
                fg = feats_pool.tile([P, FC, hw_blk], mm_dt)
                nc.sync.dma_start(out=fg, in_=feats_r[:, g])
                fgs.append(fg)

            ones_f = consts.tile([P, 2], F32)
            nc.vector.memset(ones_f, 1.0)

            bias_ap = bias_d.ap()
            bias_bc = consts.tile([K, E], F32)
            nc.gpsimd.dma_start(
                out=bias_bc,
                in_=bass.AP(
                    tensor=bias_ap.tensor, offset=bias_ap.offset, ap=[[0, K], [1, E]]
                ),
            )

            # psum_out columns [0:E) accumulate onehot.T @ projT; columns
            # [E:E+2) accumulate onehot.T @ 1 = the class sizes.
            psum_out = ps_out_pool.tile([K, E + 2], F32)
            oh_all = consts.tile([P, n_t, K], mm_dt)

            # Phase 1 (DVE only): rowmax + is_equal per 128-pixel chunk.
            for t in range(n_t):
                rowmax = small.tile([P, 1], F32)
                nc.vector.tensor_reduce(
                    rowmax, outputs_sb[:, t, :], mybir.AxisListType.X,
                    mybir.AluOpType.max,
                )
                nc.vector.tensor_scalar(
                    out=oh_all[:, t, :],
                    in0=outputs_sb[:, t, :],
                    scalar1=rowmax,
                    scalar2=None,
                    op0=mybir.AluOpType.is_equal,
                )

            for g in range(n_blk):
                fg = fgs[g]
                for s in range(sub):
                    t = g * sub + s
                    pt = ps_proj.tile([P, E], F32)
                    for fc in range(FC):
                        nc.tensor.matmul(
                            pt,
                            lhsT=fg[:, fc, ts(s, P)],
                            rhs=wT_sb[:, fc, :],
                            start=(fc == 0),
                            stop=(fc == FC - 1),
                        )
                    pts = projT_pool.tile([P, E + 2], mm_dt)
                    nc.vector.tensor_copy(pts[:, E : E + 2], ones_f)
                    nc.vector.tensor_copy(pts[:, 0:E], pt)
                    nc.tensor.matmul(
                        psum_out,
                        lhsT=oh_all[:, t, :],
                        rhs=pts,
                        start=(t == 0),
                        stop=(t == n_t - 1),
                    )

            # Phase 3: scale by 1/sizes, add bias, store
            sizes_sb = small.tile([K, 1], F32, tag="sizes")
            nc.vector.tensor_scalar_add(sizes_sb, psum_out[:, E : E + 1], 0.01)
            recip = small.tile([K, 1], F32, tag="recip")
            nc.vector.reciprocal(recip, sizes_sb)
            out_sb = outp.tile([K, E], F32)
            nc.vector.scalar_tensor_tensor(
                out=out_sb,
                in0=psum_out[:, 0:E],
                scalar=recip,
                in1=bias_bc,
                op0=mybir.AluOpType.mult,
                op1=mybir.AluOpType.add,
            )
            nc.sync.dma_start(out=out_d.ap(), in_=out_sb)

    nc.compile()
    return nc


_CACHE = {}


def make_in_maps(outputs, feats, w_proj, b_proj, dtype=DTYPE):
    import ml_dtypes

    mm_np = ml_dtypes.bfloat16 if dtype == "bf16" else np.float32
    outputs = np.asarray(outputs, dtype=np.float32)
    # [B, K, H, W] -> per sample [p, t, k] (pixel-major: hw = t*128 + p)
    outputs_t = np.ascontiguousarray(
        outputs.reshape(B, K, HW // P, P).transpose(0, 3, 2, 1)
    )
    feats = np.asarray(feats, dtype=np.float32).astype(mm_np)
    # [B, F, H, W] -> per sample [p, g, fc, hw_blk] block-major layout
    feats_sh = np.ascontiguousarray(
        feats.reshape(B, FC, P, N_BLK, HW_BLK).transpose(0, 2, 3, 1, 4)
    )
    wT = np.ascontiguousarray(np.asarray(w_proj, dtype=np.float32).T.astype(mm_np))
    bias = np.ascontiguousarray(np.asarray(b_proj, dtype=np.float32))
    return [
        {
            "outputs_in": outputs_t[b],
            "feats_in": feats_sh[b],
            "wT_in": wT,
            "bias_in": bias,
        }
        for b in range(B)
    ]


def kernel(outputs, feats, w_proj, b_proj, _trace=False, _trace_kwargs=None,
           _dtype=DTYPE, _build_kwargs=None):
    key = (_dtype, tuple(sorted((_build_kwargs or {}).items())))
    if key not in _CACHE:
        _CACHE[key] = build_module(dtype=_dtype, **(_build_kwargs or {}))
    nc = _CACHE[key]
    in_maps = make_in_maps(outputs, feats, w_proj, b_proj, dtype=_dtype)
    res = run_bass_kernel_spmd(
        nc,
        in_maps,
        core_ids=list(range(N_CORES)),
        trace=_trace,
        **(_trace_kwargs or {}),
    )
    out = np.stack([np.asarray(r["out"]).T for r in res.results])
    if _trace:
        _CACHE["last_results"] = res
    return out


# revision 37
# speedup vs baseline: 1.5296x; 1.3013x over previous
"""Trainium2 Bass kernel for nn_Encoder segment-reduce.

Reference computation (per sample b):
    cls = onehot(argmax_k outputs[b])            # [K, HW]
    sizes = cls.sum(HW) + 0.01                   # [K]
    feat_set = feats[b] @ cls.T / sizes          # [F, K]
    out[b] = w_proj @ feat_set + bias            # [E, K]

Kernel strategy (pure data parallel: 1 sample per NeuronCore, 8 cores).

Segment-reduce FIRST (the cheap contraction), projection second:
    feat_setT[k, f] = sum_hw onehot[hw, k] * featsT[hw, f]
computed with the onehot chunk [128hw, 21] as the PE's stationary operand and
featsT chunks [128hw, 512f] as the moving operand, accumulating four [21, 512]
PSUM tiles across all 32 hw chunks.  This streams feats through the PE exactly
once (65K cycles) — the minimum possible — so the kernel is DMA-bound.
A parallel [21, 2] PSUM tile accumulates onehot.T @ ones = the class sizes.

The host supplies:
  - outputs pixel-major [p, t, k] so the argmax is a free-dim reduce (DVE)
    with no PE transposes;
  - featsT block-major [p, t4, fgrp, 512] (a pure layout permutation of the
    bf16-cast feats) so each partition's per-block DMA run is 8KB contiguous.

After the stream: scale rows by 1/sizes, PE-transpose the [21, 2048] result
back to f-major in 128-col chunks, and apply the (tiny) w_proj projection +
bias, writing [E, K] directly.

A burst of dummy matmuls at kernel start keeps the PE's HAM clock gate warm
through the initial DMA window (cold PE runs at 1.2 GHz vs 2.4 GHz warm).

dtype: "bf16" (rel err ~3e-3, half HBM traffic) or "f32r" (float32r full-rate
fp32 matmuls, rel err ~2e-4, double the traffic).
"""

import numpy as np

import concourse.bacc as bacc
import concourse.bass as bass
import concourse.mybir as mybir
import concourse.tile as tile
from concourse.bass import ds, ts
from concourse.bass_utils import run_bass_kernel_spmd
from concourse.masks import make_identity

# Problem shapes (hardcoded per contract)
B = 8
K = 21
H = 64
W = 64
HW = H * W            # 4096
F = 2048
E = 256
P = 128
FC = F // P           # 16 f-chunks of 128
FG = 4                # f-groups of 512 (psum accumulate tiles)
FGW = F // FG         # 512
N_T = HW // P         # 32 hw chunks
TB = 4                # hw chunks per DMA block
N_BLK = N_T // TB     # 8 blocks (2MB bf16 each)
N_CORES = 8

F32 = mybir.dt.float32
F32R = mybir.dt.float32r
BF16 = mybir.dt.bfloat16

DTYPE = "bf16"        # "bf16" or "f32r"


def build_module(dtype=DTYPE, feats_bufs=6, warmup=170):
    mm_dt = BF16 if dtype == "bf16" else F32R
    nc = bacc.Bacc("TRN2", target_bir_lowering=False, debug=False)

    # outputs host-transposed to [p, t, k] (pixel-major).
    outputs_d = nc.dram_tensor("outputs_in", [P, N_T, K], F32, kind="ExternalInput")
    # featsT host-permuted to [p, t, fgrp, fj]: featsT[t*128+p, fgrp*512+fj].
    feats_d = nc.dram_tensor(
        "feats_in", [P, N_T, FG, FGW], mm_dt, kind="ExternalInput"
    )
    wT_d = nc.dram_tensor("wT_in", [F, E], mm_dt, kind="ExternalInput")
    bias_d = nc.dram_tensor("bias_in", [E], F32, kind="ExternalInput")
    out_d = nc.dram_tensor("out", [E, K], F32, kind="ExternalOutput")

    with tile.TileContext(nc) as tc:
        with (
            tc.tile_pool(name="consts", bufs=1) as consts,
            tc.tile_pool(name="feats", bufs=feats_bufs) as feats_pool,
            tc.tile_pool(name="small", bufs=4) as small,
            tc.tile_pool(name="outp", bufs=1) as outp,
            tc.tile_pool(name="ps_fs", bufs=1, space="PSUM") as ps_fs,
            tc.tile_pool(name="ps_sz", bufs=1, space="PSUM") as ps_sz,
            tc.tile_pool(name="ps_misc", bufs=3, space="PSUM") as ps_misc,
        ):
            # Bulk DMAs in FIFO order on the sync HWDGE queue: outputs first
            # (phase 1), then the featsT block stream.  wT/bias ride the
            # gpsimd SWDGE queue in parallel (needed only at the tail).
            outputs_sb = consts.tile([P, N_T, K], F32)
            nc.sync.dma_start(out=outputs_sb, in_=outputs_d.ap())

            feats_r = feats_d.ap()
            fgs = []
            for g in range(N_BLK):
                fg = feats_pool.tile([P, TB, FG, FGW], mm_dt)
                nc.sync.dma_start(out=fg, in_=feats_r[:, ds(g * TB, TB)])
                fgs.append(fg)

            wT_sb = consts.tile([P, FC, E], mm_dt)
            nc.gpsimd.dma_start(
                out=wT_sb, in_=wT_d.ap().rearrange("(fc p) e -> p fc e", p=P)
            )
            bias_sb = consts.tile([P, 2], F32)
            nc.gpsimd.dma_start(
                out=bias_sb, in_=bias_d.ap().rearrange("(ec p) -> p ec", p=P)
            )

            # PE warm-up: HAM holds the PE at 1.2 GHz until ~3.4us of
            # sustained activity; dummy matmuls bridge the initial DMA wait.
            warm_w = consts.tile([P, 64], BF16)
            nc.vector.memset(warm_w, 0.0)
            warm_ps = ps_misc.tile([P, 64], F32, tag="m")
            for _ in range(warmup):
                nc.tensor.matmul(warm_ps[0:64, :], lhsT=warm_w, rhs=warm_w)

            ident = consts.tile([P, P], F32)
            make_identity(nc, ident)
            ones_b = consts.tile([P, 2], mm_dt)
            nc.vector.memset(ones_b, 1.0)

            # Phase 1 (DVE only): onehot = (outT == rowmax) per hw chunk.
            oh_all = consts.tile([P, N_T, K], mm_dt)
            for t in range(N_T):
                rowmax = small.tile([P, 1], F32)
                nc.vector.tensor_reduce(
                    rowmax, outputs_sb[:, t, :], mybir.AxisListType.X,
                    mybir.AluOpType.max,
                )
                nc.vector.tensor_scalar(
                    out=oh_all[:, t, :],
                    in0=outputs_sb[:, t, :],
                    scalar1=rowmax,
                    scalar2=None,
                    op0=mybir.AluOpType.is_equal,
                )

            # Segment-reduce stream: feat_setT[k, f] and sizes accumulate in
            # PSUM across all 32 hw chunks; feats passes the PE exactly once.
            fs_ps = [
                ps_fs.tile([K, FGW], F32, name=f"fs{i}", tag=f"fs{i}")
                for i in range(FG)
            ]
            sz_ps = ps_sz.tile([K, 2], F32)
            for g in range(N_BLK):
                fg = fgs[g]
                for ti in range(TB):
                    t = g * TB + ti
                    oh_t = oh_all[:, t, :]
                    for fgrp in range(FG):
                        nc.tensor.matmul(
                            fs_ps[fgrp],
                            lhsT=oh_t,
                            rhs=fg[:, ti, fgrp, :],
                            start=(t == 0),
                            stop=(t == N_T - 1),
                        )
                    nc.tensor.matmul(
                        sz_ps,
                        lhsT=oh_t,
                        rhs=ones_b,
                        start=(t == 0),
                        stop=(t == N_T - 1),
                    )

            # Tail: divide by sizes, transpose feat_set back to f-major,
            # project with w_proj, add bias, store [E, K].
            sizes_sb = small.tile([K, 1], F32, tag="sizes")
            nc.vector.tensor_scalar_add(sizes_sb, sz_ps[:, 0:1], 0.01)
            recip = small.tile([K, 1], F32, tag="recip")
            nc.vector.reciprocal(recip, sizes_sb)

            fs_sc = consts.tile([K, F], F32)
            for fgrp in range(FG):
                nc.vector.tensor_scalar_mul(
                    fs_sc[:, ds(fgrp * FGW, FGW)], fs_ps[fgrp], recip
                )

            fsT_sb = consts.tile([P, FC, K], mm_dt)
            ps_o = [None, None]
            out_sb = outp.tile([P, 2, K], F32)
            for fc in range(FC):
                trp = ps_misc.tile([P, K], F32, tag="m")
                nc.tensor.transpose(trp, fs_sc[:, ts(fc, P)], ident[:K, :K])
                nc.vector.tensor_copy(fsT_sb[:, fc, :], trp)
            for ec in range(2):
                ps_o_ec = ps_misc.tile([P, K], F32, tag="m")
                ps_o[ec] = ps_o_ec
                for fc in range(FC):
                    nc.tensor.matmul(
                        ps_o[ec],
                        lhsT=wT_sb[:, fc, ds(ec * P, P)],
                        rhs=fsT_sb[:, fc, :],
                        start=(fc == 0),
                        stop=(fc == FC - 1),
                    )
                nc.vector.tensor_scalar_add(
                    out_sb[:, ec, :], ps_o[ec], bias_sb[:, ec : ec + 1]
                )
            nc.sync.dma_start(
                out=out_d.ap().rearrange("(ec p) k -> p ec k", p=P), in_=out_sb
            )

    nc.compile()
    return nc


_CACHE = {}


def make_in_maps(outputs, feats, w_proj, b_proj, dtype=DTYPE):
    import ml_dtypes

    mm_np = ml_dtypes.bfloat16 if dtype == "bf16" else np.float32
    outputs = np.asarray(outputs, dtype=np.float32)
    # [B, K, H, W] -> per sample [p, t, k] (pixel-major: hw = t*128 + p)
    outputs_t = np.ascontiguousarray(
        outputs.reshape(B, K, N_T, P).transpose(0, 3, 2, 1)
    )
    feats = np.asarray(feats, dtype=np.float32).astype(mm_np)
    # [B, F, H, W] -> per sample [p, t, fgrp, fj] = featsT[t*128+p, fgrp*512+fj]
    feats_sh = np.ascontiguousarray(
        feats.reshape(B, FG, FGW, N_T, P).transpose(0, 4, 3, 1, 2)
    )
    wT = np.ascontiguousarray(np.asarray(w_proj, dtype=np.float32).T.astype(mm_np))
    bias = np.ascontiguousarray(np.asarray(b_proj, dtype=np.float32))
    return [
        {
            "outputs_in": outputs_t[b],
            "feats_in": feats_sh[b],
            "wT_in": wT,
            "bias_in": bias,
        }
        for b in range(B)
    ]


def kernel(outputs, feats, w_proj, b_proj, _trace=False, _trace_kwargs=None,
           _dtype=DTYPE, _build_kwargs=None):
    key = (_dtype, tuple(sorted((_build_kwargs or {}).items())))
    if key not in _CACHE:
        _CACHE[key] = build_module(dtype=_dtype, **(_build_kwargs or {}))
    nc = _CACHE[key]
    in_maps = make_in_maps(outputs, feats, w_proj, b_proj, dtype=_dtype)
    res = run_bass_kernel_spmd(
        nc,
        in_maps,
        core_ids=list(range(N_CORES)),
        trace=_trace,
        **(_trace_kwargs or {}),
    )
    out = np.stack([np.asarray(r["out"]) for r in res.results])
    if _trace:
        _CACHE["last_results"] = res
    return out


# revision 39
# speedup vs baseline: 1.5489x; 1.0126x over previous
"""Trainium2 Bass kernel for nn_Encoder segment-reduce.

Reference computation (per sample b):
    cls = onehot(argmax_k outputs[b])            # [K, HW]
    sizes = cls.sum(HW) + 0.01                   # [K]
    feat_set = feats[b] @ cls.T / sizes          # [F, K]
    out[b] = w_proj @ feat_set + bias            # [E, K]

Kernel strategy (pure data parallel: 1 sample per NeuronCore, 8 cores).

Segment-reduce FIRST (the cheap contraction), projection second:
    feat_setT[k, f] = sum_hw onehot[hw, k] * featsT[hw, f]
computed with the onehot chunk [128hw, 21] as the PE's stationary operand and
featsT chunks [128hw, 512f] as the moving operand, accumulating four [21, 512]
PSUM tiles across all 32 hw chunks.  This streams feats through the PE exactly
once (65K cycles) — the minimum possible — so the kernel is DMA-bound.
A parallel [21, 2] PSUM tile accumulates onehot.T @ ones = the class sizes.

The host supplies:
  - outputs pixel-major [p, t, k] so the argmax is a free-dim reduce (DVE)
    with no PE transposes;
  - featsT block-major [p, t4, fgrp, 512] (a pure layout permutation of the
    bf16-cast feats) so each partition's per-block DMA run is 8KB contiguous.

After the stream: scale rows by 1/sizes, PE-transpose the [21, 2048] result
back to f-major in 128-col chunks, and apply the (tiny) w_proj projection +
bias, writing [E, K] directly.

A burst of dummy matmuls at kernel start keeps the PE's HAM clock gate warm
through the initial DMA window (cold PE runs at 1.2 GHz vs 2.4 GHz warm).

dtype: "bf16" (rel err ~3e-3, half HBM traffic) or "f32r" (float32r full-rate
fp32 matmuls, rel err ~2e-4, double the traffic).
"""

import numpy as np

import concourse.bacc as bacc
import concourse.bass as bass
import concourse.mybir as mybir
import concourse.tile as tile
from concourse.bass import ds, ts
from concourse.bass_utils import run_bass_kernel_spmd
from concourse.masks import make_identity

# Problem shapes (hardcoded per contract)
B = 8
K = 21
H = 64
W = 64
HW = H * W            # 4096
F = 2048
E = 256
P = 128
FC = F // P           # 16 f-chunks of 128
FG = 4                # f-groups of 512 (psum accumulate tiles)
FGW = F // FG         # 512
N_T = HW // P         # 32 hw chunks
TB = 4                # hw chunks per DMA block
N_BLK = N_T // TB     # 8 blocks (2MB bf16 each)
N_CORES = 8

F32 = mybir.dt.float32
F32R = mybir.dt.float32r
BF16 = mybir.dt.bfloat16

DTYPE = "bf16"        # "bf16" or "f32r"


def build_module(dtype=DTYPE, feats_bufs=6, warmup=100):
    mm_dt = BF16 if dtype == "bf16" else F32R
    nc = bacc.Bacc("TRN2", target_bir_lowering=False, debug=False)

    # outputs host-transposed to [p, t, k] (pixel-major).
    outputs_d = nc.dram_tensor("outputs_in", [P, N_T, K], F32, kind="ExternalInput")
    # featsT host-permuted to [p, t, fgrp, fj]: featsT[t*128+p, fgrp*512+fj].
    feats_d = nc.dram_tensor(
        "feats_in", [P, N_T, FG, FGW], mm_dt, kind="ExternalInput"
    )
    wT_d = nc.dram_tensor("wT_in", [F, E], mm_dt, kind="ExternalInput")
    bias_d = nc.dram_tensor("bias_in", [E], F32, kind="ExternalInput")
    out_d = nc.dram_tensor("out", [E, K], F32, kind="ExternalOutput")

    with tile.TileContext(nc) as tc:
        with (
            tc.tile_pool(name="consts", bufs=1) as consts,
            tc.tile_pool(name="feats", bufs=feats_bufs) as feats_pool,
            tc.tile_pool(name="small", bufs=4) as small,
            tc.tile_pool(name="outp", bufs=1) as outp,
            tc.tile_pool(name="ps_fs", bufs=1, space="PSUM") as ps_fs,
            tc.tile_pool(name="ps_sz", bufs=1, space="PSUM") as ps_sz,
            tc.tile_pool(name="ps_misc", bufs=3, space="PSUM") as ps_misc,
        ):
            # Bulk DMAs in FIFO order on the sync HWDGE queue: outputs first
            # (phase 1), then the featsT block stream.  wT/bias ride the
            # gpsimd SWDGE queue in parallel (needed only at the tail).
            outputs_sb = consts.tile([P, N_T, K], F32)
            nc.sync.dma_start(out=outputs_sb, in_=outputs_d.ap())

            feats_r = feats_d.ap()
            fgs = []
            for g in range(N_BLK):
                fg = feats_pool.tile([P, TB, FG, FGW], mm_dt)
                nc.sync.dma_start(out=fg, in_=feats_r[:, ds(g * TB, TB)])
                fgs.append(fg)

            wT_sb = consts.tile([P, FC, E], mm_dt)
            nc.gpsimd.dma_start(
                out=wT_sb, in_=wT_d.ap().rearrange("(fc p) e -> p fc e", p=P)
            )
            bias_sb = consts.tile([P, 2], F32)
            nc.gpsimd.dma_start(
                out=bias_sb, in_=bias_d.ap().rearrange("(ec p) -> p ec", p=P)
            )

            # PE warm-up: HAM holds the PE at 1.2 GHz until ~3.4us of
            # sustained activity; dummy matmuls bridge the initial DMA wait.
            warm_w = consts.tile([P, 64], BF16)
            nc.vector.memset(warm_w, 0.0)
            warm_ps = ps_misc.tile([P, 64], F32, tag="m")
            for _ in range(warmup):
                nc.tensor.matmul(warm_ps[0:64, :], lhsT=warm_w, rhs=warm_w)

            ident = consts.tile([P, P], F32)
            make_identity(nc, ident)
            ones_b = consts.tile([P, 2], mm_dt)
            nc.vector.memset(ones_b, 1.0)

            # Phase 1 (DVE only): onehot = (outT == rowmax) per hw chunk.
            oh_all = consts.tile([P, N_T, K], mm_dt)
            for t in range(N_T):
                rowmax = small.tile([P, 1], F32)
                nc.vector.tensor_reduce(
                    rowmax, outputs_sb[:, t, :], mybir.AxisListType.X,
                    mybir.AluOpType.max,
                )
                nc.vector.tensor_scalar(
                    out=oh_all[:, t, :],
                    in0=outputs_sb[:, t, :],
                    scalar1=rowmax,
                    scalar2=None,
                    op0=mybir.AluOpType.is_equal,
                )

            # Class sizes: onehot.T @ ones, all chunks upfront (only needs
            # oh_all) so the reciprocal is ready before the stream ends.
            sz_ps = ps_sz.tile([K, 2], F32)
            for t in range(N_T):
                nc.tensor.matmul(
                    sz_ps,
                    lhsT=oh_all[:, t, :],
                    rhs=ones_b,
                    start=(t == 0),
                    stop=(t == N_T - 1),
                )
            sizes_sb = small.tile([K, 1], F32, tag="sizes")
            nc.vector.tensor_scalar_add(sizes_sb, sz_ps[:, 0:1], 0.01)
            recip = small.tile([K, 1], F32, tag="recip")
            nc.vector.reciprocal(recip, sizes_sb)

            # Segment-reduce stream: feat_setT[k, f] accumulates in PSUM
            # across all 32 hw chunks; feats passes the PE exactly once.
            fs_ps = [
                ps_fs.tile([K, FGW], F32, name=f"fs{i}", tag=f"fs{i}")
                for i in range(FG)
            ]
            for g in range(N_BLK):
                fg = fgs[g]
                for ti in range(TB):
                    t = g * TB + ti
                    oh_t = oh_all[:, t, :]
                    for fgrp in range(FG):
                        nc.tensor.matmul(
                            fs_ps[fgrp],
                            lhsT=oh_t,
                            rhs=fg[:, ti, fgrp, :],
                            start=(t == 0),
                            stop=(t == N_T - 1),
                        )

            # Tail: divide by sizes (fused into the PSUM->SBUF copies, split
            # across DVE and ACT), transpose feat_set back to f-major,
            # project with w_proj, add bias, store [E, K].
            fs_sc = consts.tile([K, F], F32)
            for fgrp in range(FG):
                if fgrp % 2 == 0:
                    nc.vector.tensor_scalar_mul(
                        fs_sc[:, ds(fgrp * FGW, FGW)], fs_ps[fgrp], recip
                    )
                else:
                    nc.scalar.activation(
                        out=fs_sc[:, ds(fgrp * FGW, FGW)],
                        in_=fs_ps[fgrp],
                        func=mybir.ActivationFunctionType.Copy,
                        scale=recip,
                    )

            fsT_sb = consts.tile([P, FC, K], mm_dt)
            ps_o = [None, None]
            out_sb = outp.tile([P, 2, K], F32)
            for fc in range(FC):
                trp = ps_misc.tile([P, K], F32, tag="m")
                nc.tensor.transpose(trp, fs_sc[:, ts(fc, P)], ident[:K, :K])
                nc.vector.tensor_copy(fsT_sb[:, fc, :], trp)
            for ec in range(2):
                ps_o_ec = ps_misc.tile([P, K], F32, tag="m")
                ps_o[ec] = ps_o_ec
                for fc in range(FC):
                    nc.tensor.matmul(
                        ps_o[ec],
                        lhsT=wT_sb[:, fc, ds(ec * P, P)],
                        rhs=fsT_sb[:, fc, :],
                        start=(fc == 0),
                        stop=(fc == FC - 1),
                    )
                nc.vector.tensor_scalar_add(
                    out_sb[:, ec, :], ps_o[ec], bias_sb[:, ec : ec + 1]
                )
            nc.sync.dma_start(
                out=out_d.ap().rearrange("(ec p) k -> p ec k", p=P), in_=out_sb
            )

    nc.compile()
    return nc


_CACHE = {}


def make_in_maps(outputs, feats, w_proj, b_proj, dtype=DTYPE):
    import ml_dtypes

    mm_np = ml_dtypes.bfloat16 if dtype == "bf16" else np.float32
    outputs = np.asarray(outputs, dtype=np.float32)
    # [B, K, H, W] -> per sample [p, t, k] (pixel-major: hw = t*128 + p)
    outputs_t = np.ascontiguousarray(
        outputs.reshape(B, K, N_T, P).transpose(0, 3, 2, 1)
    )
    feats = np.asarray(feats, dtype=np.float32).astype(mm_np)
    # [B, F, H, W] -> per sample [p, t, fgrp, fj] = featsT[t*128+p, fgrp*512+fj]
    feats_sh = np.ascontiguousarray(
        feats.reshape(B, FG, FGW, N_T, P).transpose(0, 4, 3, 1, 2)
    )
    wT = np.ascontiguousarray(np.asarray(w_proj, dtype=np.float32).T.astype(mm_np))
    bias = np.ascontiguousarray(np.asarray(b_proj, dtype=np.float32))
    return [
        {
            "outputs_in": outputs_t[b],
            "feats_in": feats_sh[b],
            "wT_in": wT,
            "bias_in": bias,
        }
        for b in range(B)
    ]


def kernel(outputs, feats, w_proj, b_proj, _trace=False, _trace_kwargs=None,
           _dtype=DTYPE, _build_kwargs=None):
    key = (_dtype, tuple(sorted((_build_kwargs or {}).items())))
    if key not in _CACHE:
        _CACHE[key] = build_module(dtype=_dtype, **(_build_kwargs or {}))
    nc = _CACHE[key]
    in_maps = make_in_maps(outputs, feats, w_proj, b_proj, dtype=_dtype)
    res = run_bass_kernel_spmd(
        nc,
        in_maps,
        core_ids=list(range(N_CORES)),
        trace=_trace,
        **(_trace_kwargs or {}),
    )
    out = np.stack([np.asarray(r["out"]) for r in res.results])
    if _trace:
        _CACHE["last_results"] = res
    return out
